# revision 24
# baseline (speedup 1.0000x reference)
"""Multi-head attention block (QKV proj -> softmax attention -> out proj) for
Trainium2, SPMD across 8 NeuronCores.

Sharding: batch (B=2) x head-groups (4 groups of 4 heads). Core c handles
batch c//4 and heads [4*(c%4), 4*(c%4)+4). Each core computes its partial
output contribution (context @ wo_slice.T); the host sums the 4 head-group
partials per batch (tensor-parallel row-sharded wo => the all-reduce is the
host-side gather).

All matmuls run in bf16 with fp32 PSUM accumulation. Softmax runs in fp32
out of PSUM (exp on the scalar engine). Softmax denominators are computed by
accumulating the exp'd probability tiles on the DVE (fp32) and reducing the
final [128, 512] accumulator over partitions with a single ones-vector
matmul per unit -- this keeps the PE free of the per-tile ones-matmuls that
cost ~78us in v1. The reciprocal runs on a [128, 4] layout (128 partitions
busy) instead of [1, 512] (one partition). Partial outputs are written fp16
(host accumulates in fp32).

Per-core kernel layout (everything [partition=128, free]):
  xT   [2048, 2048] bf16   x[b].T             (feature k on partitions)
  wqT/wkT/wvT [2048, 512]  w[heads_slice].T   (k on partitions)
  woT  [512, 2048]  bf16   wo[:, slice].T     (local d on partitions)
  out  [2048, 2048] fp16   partial output for batch b
"""

import sys

if "/opt/trn_rl_repo" not in sys.path:
    sys.path.insert(0, "/opt/trn_rl_repo")

from contextlib import ExitStack

import ml_dtypes
import numpy as np

import concourse.bacc as bacc
import concourse.tile as tile
from concourse import mybir
from concourse.bass_utils import run_bass_kernel_spmd

BF16 = mybir.dt.bfloat16
F16 = mybir.dt.float16
F32 = mybir.dt.float32

B, S, DIM = 2, 2048, 2048
HEADS, HD = 16, 128
P = 128
N_CORES = 8
HGROUPS = 4  # head groups (second shard axis is batch)
HPC = HEADS // HGROUPS  # heads per core = 4
DL = HPC * HD  # local head dims per core = 512
SCALE = 1.0 / float(np.sqrt(HD))

NK = DIM // P  # 16 contraction tiles for the projections
NM = S // 512  # 4 m-chunks (tokens)
NQ = S // P  # 16 q tiles
NN = S // P  # 16 kv tiles
NE = DIM // 512  # 4 output-dim chunks

_PROGRAM_CACHE = {}


def _emit(nc, tc, xT, wqT, wkT, wvT, woT, maskf, out):
    with_mask = maskf is not None
    with ExitStack() as octx:
        planes = octx.enter_context(tc.tile_pool(name="planes", bufs=1))
        q_sb = [planes.tile([P, S], BF16, tag=f"q{h}", name=f"q{h}") for h in range(HPC)]
        k_sb = [planes.tile([P, S], BF16, tag=f"k{h}", name=f"k{h}") for h in range(HPC)]
        ctx_sb = [planes.tile([P, S], F16, tag=f"ctx{h}", name=f"ctx{h}") for h in range(HPC)]

        vv_pool = octx.enter_context(tc.tile_pool(name="vv", bufs=1))
        vvs = [None] * HPC

        # ---------------- Phase 1: QKV projections ----------------
        with ExitStack() as ctx:
            wpool = ctx.enter_context(tc.tile_pool(name="wqkv", bufs=1))
            vT_sb = [wpool.tile([P, S], F16, tag=f"vt{h}", name=f"vt{h}")
                     for h in range(HPC)]
            w_sb = {}
            for name in ("q", "k", "v"):
                w_sb[name] = wpool.tile([P, NK * DL], BF16, tag=f"w{name}",
                                        name=f"w{name}")
            xpool = ctx.enter_context(tc.tile_pool(name="xt", bufs=2 * NK))
            pq = ctx.enter_context(tc.tile_pool(name="ps_qkv", bufs=4, space="PSUM"))

            for mc in range(NM):
                if mc == 0:
                    # each weight plane as one large DMA on the gpsimd queue,
                    # in parallel with the x tiles (split sync/scalar)
                    for name, srct in (("k", wkT), ("v", wvT), ("q", wqT)):
                        nc.gpsimd.dma_start(
                            w_sb[name][:].rearrange("p (k c) -> p k c", k=NK),
                            srct[:].rearrange("(k p) c -> p k c", p=P),
                        )
                xts = []
                for kt in range(NK):
                    t = xpool.tile([P, 512], BF16, tag="xt")
                    xq = nc.sync if kt % 2 == 0 else nc.scalar
                    xq.dma_start(
                        t[:], xT[kt * P : (kt + 1) * P, mc * 512 : (mc + 1) * 512]
                    )
                    xts.append(t)
                # k/v planes first so attention for head h can start (and the
                # vv transposes can issue) before the q planes of the last
                # m-chunk are finished
                plan = [(n, h) for h in range(HPC) for n in ("k", "v")]
                plan += [("q", h) for h in range(HPC)]
                for name, h in plan:
                    plane = {"q": q_sb, "k": k_sb, "v": vT_sb}[name][h]
                    ps = pq.tile([P, 512], F32, tag="ps")
                    for kt in range(NK):
                        nc.tensor.matmul(
                            ps[:],
                            w_sb[name][:, kt * DL + h * P : kt * DL + (h + 1) * P],
                            xts[kt][:],
                            start=(kt == 0),
                            stop=(kt == NK - 1),
                        )
                    nc.any.tensor_copy(plane[:, mc * 512 : (mc + 1) * 512], ps[:])
                    if name == "v" and mc == NM - 1:
                        # v plane complete: transpose to [kv, d] via DMA x-bar
                        vv = vv_pool.tile([P, NN, P], F16, tag=f"vv{h}",
                                          name=f"vv{h}")
                        nc.sync.dma_start(vv[:], vT_sb[h][:], transpose=True)
                        vvs[h] = vv

        # ------- Phase 2+3: attention (scoresT form) + out projection -------
        # scoresT(nt) = kT(nt)^T @ qT  ->  exp -> probsT [n, m] directly (no
        # transpose on the softmax path). Denominators: DVE accumulates the
        # exp'd tiles (fp32), one small ones-matmul per unit reduces over
        # partitions; normalization applied to the small context tensor
        # through a pair of x-bar transposes, off the critical path.
        with ExitStack() as ctx:
            wopool = ctx.enter_context(tc.tile_pool(name="wo", bufs=1))
            wo_sb = [wopool.tile([P, DIM], F16, tag=f"wo{h}", name=f"wo{h}")
                     for h in range(HPC)]
            for h in range(HPC):
                nc.gpsimd.dma_start(wo_sb[h][:], woT[h * P : (h + 1) * P, :])

            const = ctx.enter_context(tc.tile_pool(name="const", bufs=1))
            ones_col = const.tile([P, 1], F16, tag="ones_col")
            nc.any.memset(ones_col[:], 1.0)
            ones_one = const.tile([1, 1], F16, tag="ones_one")
            nc.any.memset(ones_one[:], 1.0)

            pbt_pool = ctx.enter_context(tc.tile_pool(name="pbt", bufs=3))
            accw_pool = ctx.enter_context(tc.tile_pool(name="accw", bufs=3))
            ctmp_pool = ctx.enter_context(tc.tile_pool(name="ctmp", bufs=3))
            cnrm_pool = ctx.enter_context(tc.tile_pool(name="cnrm", bufs=3))
            stats = ctx.enter_context(tc.tile_pool(name="stats", bufs=4))
            opool = ctx.enter_context(tc.tile_pool(name="ob", bufs=3))
            if with_mask:
                mpool = ctx.enter_context(tc.tile_pool(name="mask", bufs=4))
            ps_s = ctx.enter_context(tc.tile_pool(name="ps_s", bufs=2, space="PSUM"))
            ps_pv = ctx.enter_context(tc.tile_pool(name="ps_pv", bufs=2, space="PSUM"))
            ps_misc = ctx.enter_context(
                tc.tile_pool(name="ps_misc", bufs=2, space="PSUM")
            )

            def d_group(tt, ec):
                ps = ps_misc.tile([P, 512], F32, tag="ps_misc")
                for h in range(HPC):
                    nc.tensor.matmul(
                        ps[:],
                        ctx_sb[h][:, tt * P : (tt + 1) * P],
                        wo_sb[h][:, ec * 512 : (ec + 1) * 512],
                        start=(h == 0),
                        stop=(h == HPC - 1),
                    )
                ob = opool.tile([P, 512], F16, tag="ob")
                # PSUM evacuation on the scalar engine: its FIFO runs the
                # copy right after the neighboring exp, freeing the bank on a
                # deterministic schedule (the DVE queue has jittery backlog)
                nc.scalar.copy(ob[:], ps[:])
                # out DMAs on gpsimd; the tail groups (after the last unit)
                # alternate gpsimd/scalar so the final drain uses two queues
                # (sync stays dedicated to the x-bar transposes)
                q = nc.gpsimd if (tt < 12 or (tt + ec) % 2 == 0) else nc.scalar
                q.dma_start(
                    out[tt * P : (tt + 1) * P, ec * 512 : (ec + 1) * 512], ob[:]
                )

            def unit(jb, h, d_take, pre):
                """One (q-block, head): scoresT+exp, PV, denominators, and
                normalized context write; d_take() interleaves out-proj work,
                pre[0]/pre[1] run the previous unit's deferred normalization
                stages at pair 0/1 so their PE instructions land with other
                work queued between them."""
                qs = q_sb[h][:, jb * 512 : (jb + 1) * 512]
                pbt = pbt_pool.tile([P, NN, 512], F16, tag="pbt", name="pbt")
                pbt_flat = pbt[:].rearrange("p n m -> p (n m)")
                acc_a = accw_pool.tile([P, 1024], F16, tag="acca", name="acca")
                acc_b = accw_pool.tile([P, 1024], F16, tag="accb", name="accb")
                psc = ps_pv.tile([P, 512], F32, tag="ps_pv")

                def pv(nt):
                    nc.tensor.matmul(
                        psc[:], vvs[h][:, nt, :], pbt[:, nt, :],
                        start=(nt == 0), stop=(nt == NN - 1),
                    )

                for pair in range(8):
                    if pair == 1:
                        pre[0]()
                    elif pair == 3:
                        pre[1]()
                    if pair < 4:
                        d_take()
                    ps = ps_s.tile([P, 1024], F32, tag="ps_s")
                    for sub in range(2):
                        nt = 2 * pair + sub
                        nc.tensor.matmul(
                            ps[:, sub * 512 : (sub + 1) * 512],
                            k_sb[h][:, nt * P : (nt + 1) * P],
                            qs,
                            start=True,
                            stop=True,
                        )
                    if with_mask:
                        mt = mpool.tile([P, 1024], F32, tag="mt")
                        for sub in range(2):
                            nt = 2 * pair + sub
                            nc.gpsimd.dma_start(
                                mt[:, sub * 512 : (sub + 1) * 512],
                                maskf[nt * P : (nt + 1) * P,
                                      jb * 512 : (jb + 1) * 512],
                            )
                        nc.vector.tensor_add(ps[:], ps[:], mt[:])
                    pslice = pbt_flat[:, pair * 1024 : (pair + 1) * 1024]
                    nc.scalar.activation(
                        pslice,
                        ps[:],
                        mybir.ActivationFunctionType.Exp,
                        scale=SCALE,
                    )
                    # denominator accumulation: DVE takes pairs 0+1, 2, 4, 6,
                    # 7 (fast engine finishes the chain); gpsimd takes 3+5
                    # via a 3-address add (its copies are slow, adds are ok)
                    if pair == 1:
                        p0 = pbt_flat[:, 0:1024]
                        nc.vector.tensor_add(acc_a[:], p0, pslice)
                    elif pair == 5:
                        p3 = pbt_flat[:, 3 * 1024 : 4 * 1024]
                        nc.gpsimd.tensor_add(acc_b[:], p3, pslice)
                    elif pair in (2, 4, 6, 7):
                        nc.vector.tensor_add(acc_a[:], acc_a[:], pslice)
                    if pair >= 1:
                        pv(2 * (pair - 1))
                        pv(2 * (pair - 1) + 1)
                pv(14)
                pv(15)

                # fold the two [P, 1024] accumulators to [P, 512] fp16; free
                # the PV bank via the ctmp copy promptly. The partition
                # reduction (one small matmul) and the rest of the
                # normalization are deferred into finish(), a unit later, so
                # the PE's in-order queue never blocks on this chain.
                acc_bs = stats.tile([P, 512], F16, tag="acc_bs")
                nc.gpsimd.tensor_add(
                    acc_bs[:], acc_b[:, 0:512], acc_b[:, 512:1024]
                )
                acc_as = stats.tile([P, 512], F16, tag="acc_as")
                nc.vector.tensor_add(
                    acc_as[:], acc_a[:, 0:512], acc_a[:, 512:1024]
                )
                acc_f16 = stats.tile([P, 512], F16, tag="acc_f16")
                nc.vector.tensor_add(acc_f16[:], acc_as[:], acc_bs[:])
                ctmp = ctmp_pool.tile([P, 512], F16, tag="ctmp", name="ctmp")
                nc.vector.tensor_copy(ctmp[:], psc[:])

                box = {}

                def fin1():
                    # partition-reduce the denominator accumulator
                    dpsf = ps_misc.tile([P, 512], F32, tag="ps_misc")
                    nc.tensor.matmul(
                        dpsf[0:1, :], ones_col[:], acc_f16[:],
                        start=True, stop=True,
                    )
                    den_f16 = stats.tile([1, 512], F16, tag="den_f16")
                    nc.vector.tensor_copy(den_f16[:], dpsf[0:1, :])
                    box["den"] = den_f16

                def fin2():
                    den_f16 = box["den"]
                    # broadcast den across partitions (4 tiny matmuls), then
                    # reciprocal on the [128, 4] layout (all partitions busy)
                    dtp = ps_misc.tile([P, 512], F32, tag="ps_misc")
                    for j in range(4):
                        nc.tensor.matmul(
                            dtp[:, j : j + 1],
                            den_f16[0:1, j * P : (j + 1) * P],
                            ones_one[:],
                            start=True,
                            stop=True,
                        )
                    rect = stats.tile([P, 4], F32, tag="rect")
                    nc.vector.tensor_copy(rect[:], dtp[:, 0:4])
                    rrec = stats.tile([P, 4], F32, tag="rrec")
                    nc.vector.reciprocal(rrec[:], rect[:])
                    cn = cnrm_pool.tile([P, 4, P], F16, tag="cnrm", name="cnrm")
                    nc.sync.dma_start(cn[:], ctmp[:], transpose=True)
                    for j in range(4):
                        nc.vector.tensor_scalar_mul(
                            cn[:, j, :], cn[:, j, :], rrec[:, j : j + 1]
                        )
                    ctx_dst = ctx_sb[h][:, jb * 512 : (jb + 1) * 512].rearrange(
                        "p (a b) -> p a b", a=4
                    )
                    nc.sync.dma_start(ctx_dst, cn[:], transpose=True)

                return fin1, fin2

            nofin = (lambda: None, lambda: None)
            pending = []
            for jb in range(4):
                # each unit runs the previous unit's deferred normalization
                # stages at its pairs 0/1, so ctx planes of block jb-1 are
                # complete before the units of jb+1 consume them via d_take
                d_list = (
                    [(tt, ec) for tt in range(4 * jb - 4, 4 * jb)
                     for ec in range(NE)]
                    if jb > 0 else []
                )
                di = [0]

                def d_take():
                    if di[0] < len(d_list):
                        d_group(*d_list[di[0]])
                        di[0] += 1

                for h in range(HPC):
                    pre = pending.pop(0) if pending else nofin
                    fins = unit(jb, h, d_take if h > 0 else (lambda: None), pre)
                    pending.append(fins)
                while di[0] < len(d_list):
                    d_group(*d_list[di[0]])
                    di[0] += 1
            for f1, f2 in pending:
                f1()
                f2()
            for tt in range(12, 16):
                for ec in range(NE):
                    d_group(tt, ec)


def _build(with_mask: bool):
    nc = bacc.Bacc("TRN2")
    xT = nc.dram_tensor("xT", [DIM, S], BF16, kind="ExternalInput")
    wqT = nc.dram_tensor("wqT", [DIM, DL], BF16, kind="ExternalInput")
    wkT = nc.dram_tensor("wkT", [DIM, DL], BF16, kind="ExternalInput")
    wvT = nc.dram_tensor("wvT", [DIM, DL], BF16, kind="ExternalInput")
    woT = nc.dram_tensor("woT", [DL, DIM], F16, kind="ExternalInput")
    maskf = (
        nc.dram_tensor("maskf", [S, S], F32, kind="ExternalInput")
        if with_mask
        else None
    )
    out = nc.dram_tensor("out", [S, DIM], F16, kind="ExternalOutput")
    with tile.TileContext(nc) as tc:
        _emit(nc, tc, xT, wqT, wkT, wvT, woT, maskf, out)
    nc.finalize()
    return nc


def _get_program(with_mask: bool):
    if with_mask not in _PROGRAM_CACHE:
        _PROGRAM_CACHE[with_mask] = _build(with_mask)
    return _PROGRAM_CACHE[with_mask]


def _prep_in_maps(x, mask, wq, wk, wv, wo, with_mask):
    bf = ml_dtypes.bfloat16
    f32 = np.float32
    xTs = [np.ascontiguousarray(x[b].T.astype(bf)) for b in range(B)]
    if with_mask:
        maskf = np.ascontiguousarray(mask[0, 0].T.astype(f32) / SCALE)
    in_maps = []
    for c in range(N_CORES):
        b = c // HGROUPS
        g = c % HGROUPS
        sl = slice(g * DL, (g + 1) * DL)
        m = {
            "xT": xTs[b],
            "wqT": np.ascontiguousarray(wq[sl, :].T.astype(bf)),
            "wkT": np.ascontiguousarray(wk[sl, :].T.astype(bf)),
            "wvT": np.ascontiguousarray(wv[sl, :].T.astype(bf)),
            "woT": np.ascontiguousarray(wo[:, sl].T.astype(np.float16)),
        }
        if with_mask:
            m["maskf"] = maskf
        in_maps.append(m)
    return in_maps


def run_sharded(x, mask, wq, wk, wv, wo, trace=False, trace_kwargs=None):
    """Run the SPMD kernel; returns (full_output, BassKernelResults)."""
    with_mask = bool(np.any(np.asarray(mask)))
    nc = _get_program(with_mask)
    in_maps = _prep_in_maps(
        np.asarray(x), np.asarray(mask), np.asarray(wq), np.asarray(wk),
        np.asarray(wv), np.asarray(wo), with_mask,
    )
    kw = {}
    if trace:
        kw["trace"] = True
        if trace_kwargs:
            kw["trace_kwargs"] = trace_kwargs
    res = run_bass_kernel_spmd(nc, in_maps, list(range(N_CORES)), **kw)
    out = np.zeros((B, S, DIM), np.float32)
    for c in range(N_CORES):
        out[c // HGROUPS] += res.results[c]["out"].astype(np.float32)
    return out, res


def kernel(**inputs):
    out, _ = run_sharded(
        inputs["x"], inputs["mask"], inputs["wq"], inputs["wk"], inputs["wv"],
        inputs["wo"],
    )
    return out


# revision 26
# speedup vs baseline: 1.0306x; 1.0306x over previous
"""Multi-head attention block (QKV proj -> softmax attention -> out proj) for
Trainium2, SPMD across 8 NeuronCores.

Sharding: batch (B=2) x head-groups (4 groups of 4 heads). Core c handles
batch c//4 and heads [4*(c%4), 4*(c%4)+4). Each core computes its partial
output contribution (context @ wo_slice.T); the host sums the 4 head-group
partials per batch (tensor-parallel row-sharded wo => the all-reduce is the
host-side gather).

All matmuls run in bf16 with fp32 PSUM accumulation. Softmax runs in fp32
out of PSUM (exp on the scalar engine). Softmax denominators are computed by
accumulating the exp'd probability tiles on the DVE (fp32) and reducing the
final [128, 512] accumulator over partitions with a single ones-vector
matmul per unit -- this keeps the PE free of the per-tile ones-matmuls that
cost ~78us in v1. The reciprocal runs on a [128, 4] layout (128 partitions
busy) instead of [1, 512] (one partition). Partial outputs are written fp16
(host accumulates in fp32).

Per-core kernel layout (everything [partition=128, free]):
  xT   [2048, 2048] bf16   x[b].T             (feature k on partitions)
  wqT/wkT/wvT [2048, 512]  w[heads_slice].T   (k on partitions)
  woT  [512, 2048]  bf16   wo[:, slice].T     (local d on partitions)
  out  [2048, 2048] fp16   partial output for batch b
"""

import sys

if "/opt/trn_rl_repo" not in sys.path:
    sys.path.insert(0, "/opt/trn_rl_repo")

from contextlib import ExitStack

import ml_dtypes
import numpy as np

import concourse.bacc as bacc
import concourse.tile as tile
from concourse import mybir
from concourse.bass_utils import run_bass_kernel_spmd

BF16 = mybir.dt.bfloat16
F16 = mybir.dt.float16
F32 = mybir.dt.float32

B, S, DIM = 2, 2048, 2048
HEADS, HD = 16, 128
P = 128
N_CORES = 8
HGROUPS = 4  # head groups (second shard axis is batch)
HPC = HEADS // HGROUPS  # heads per core = 4
DL = HPC * HD  # local head dims per core = 512
SCALE = 1.0 / float(np.sqrt(HD))

NK = DIM // P  # 16 contraction tiles for the projections
NM = S // 512  # 4 m-chunks (tokens)
NQ = S // P  # 16 q tiles
NN = S // P  # 16 kv tiles
NE = DIM // 512  # 4 output-dim chunks

_PROGRAM_CACHE = {}


def _emit(nc, tc, xT, wqT, wkT, wvT, woT, maskf, out):
    with_mask = maskf is not None
    with ExitStack() as octx:
        planes = octx.enter_context(tc.tile_pool(name="planes", bufs=1))
        q_sb = [planes.tile([P, S], BF16, tag=f"q{h}", name=f"q{h}") for h in range(HPC)]
        k_sb = [planes.tile([P, S], BF16, tag=f"k{h}", name=f"k{h}") for h in range(HPC)]
        ctx_sb = [planes.tile([P, S], F16, tag=f"ctx{h}", name=f"ctx{h}") for h in range(HPC)]

        vv_pool = octx.enter_context(tc.tile_pool(name="vv", bufs=1))
        vvs = [None] * HPC

        # ---------------- Phase 1: QKV projections ----------------
        with ExitStack() as ctx:
            wpool = ctx.enter_context(tc.tile_pool(name="wqkv", bufs=1))
            vT_sb = [wpool.tile([P, S], F16, tag=f"vt{h}", name=f"vt{h}")
                     for h in range(HPC)]
            w_sb = {}
            for name in ("q", "k", "v"):
                w_sb[name] = wpool.tile([P, NK * DL], BF16, tag=f"w{name}",
                                        name=f"w{name}")
            xpool = ctx.enter_context(tc.tile_pool(name="xt", bufs=2 * NK))
            pq = ctx.enter_context(tc.tile_pool(name="ps_qkv", bufs=4, space="PSUM"))

            for mc in range(NM):
                xts = []
                for kt in range(NK):
                    t = xpool.tile([P, 512], BF16, tag="xt")
                    nc.sync.dma_start(
                        t[:], xT[kt * P : (kt + 1) * P, mc * 512 : (mc + 1) * 512]
                    )
                    xts.append(t)
                    if mc == 0:
                        # k weights on the gpsimd queue in parallel with the
                        # x tiles on sync, so the first accumulation group
                        # starts as early as possible; v/q follow
                        nc.gpsimd.dma_start(
                            w_sb["k"][:, kt * DL : (kt + 1) * DL],
                            wkT[kt * P : (kt + 1) * P, :],
                        )
                if mc == 0:
                    for kt in range(NK):
                        for name, srct in (("v", wvT), ("q", wqT)):
                            nc.sync.dma_start(
                                w_sb[name][:, kt * DL : (kt + 1) * DL],
                                srct[kt * P : (kt + 1) * P, :],
                            )
                # k/v planes first so attention for head h can start (and the
                # vv transposes can issue) before the q planes of the last
                # m-chunk are finished
                plan = [(n, h) for h in range(HPC) for n in ("k", "v")]
                plan += [("q", h) for h in range(HPC)]
                for name, h in plan:
                    plane = {"q": q_sb, "k": k_sb, "v": vT_sb}[name][h]
                    ps = pq.tile([P, 512], F32, tag="ps")
                    for kt in range(NK):
                        nc.tensor.matmul(
                            ps[:],
                            w_sb[name][:, kt * DL + h * P : kt * DL + (h + 1) * P],
                            xts[kt][:],
                            start=(kt == 0),
                            stop=(kt == NK - 1),
                        )
                    nc.any.tensor_copy(plane[:, mc * 512 : (mc + 1) * 512], ps[:])
                    if name == "v" and mc == NM - 1:
                        # v plane complete: transpose to [kv, d] via DMA x-bar
                        vv = vv_pool.tile([P, NN, P], F16, tag=f"vv{h}",
                                          name=f"vv{h}")
                        nc.sync.dma_start(vv[:], vT_sb[h][:], transpose=True)
                        vvs[h] = vv

        # ------- Phase 2+3: attention (scoresT form) + out projection -------
        # scoresT(nt) = kT(nt)^T @ qT  ->  exp -> probsT [n, m] directly (no
        # transpose on the softmax path). Denominators: DVE accumulates the
        # exp'd tiles (fp32), one small ones-matmul per unit reduces over
        # partitions; normalization applied to the small context tensor
        # through a pair of x-bar transposes, off the critical path.
        with ExitStack() as ctx:
            wopool = ctx.enter_context(tc.tile_pool(name="wo", bufs=1))
            wo_sb = [wopool.tile([P, DIM], F16, tag=f"wo{h}", name=f"wo{h}")
                     for h in range(HPC)]
            for h in range(HPC):
                nc.gpsimd.dma_start(wo_sb[h][:], woT[h * P : (h + 1) * P, :])

            const = ctx.enter_context(tc.tile_pool(name="const", bufs=1))
            ones_col = const.tile([P, 1], F16, tag="ones_col")
            nc.any.memset(ones_col[:], 1.0)
            ones_one = const.tile([1, 1], F16, tag="ones_one")
            nc.any.memset(ones_one[:], 1.0)

            pbt_pool = ctx.enter_context(tc.tile_pool(name="pbt", bufs=3))
            accw_pool = ctx.enter_context(tc.tile_pool(name="accw", bufs=3))
            ctmp_pool = ctx.enter_context(tc.tile_pool(name="ctmp", bufs=3))
            cnrm_pool = ctx.enter_context(tc.tile_pool(name="cnrm", bufs=3))
            stats = ctx.enter_context(tc.tile_pool(name="stats", bufs=4))
            opool = ctx.enter_context(tc.tile_pool(name="ob", bufs=3))
            if with_mask:
                mpool = ctx.enter_context(tc.tile_pool(name="mask", bufs=4))
            ps_s = ctx.enter_context(tc.tile_pool(name="ps_s", bufs=2, space="PSUM"))
            ps_pv = ctx.enter_context(tc.tile_pool(name="ps_pv", bufs=2, space="PSUM"))
            ps_misc = ctx.enter_context(
                tc.tile_pool(name="ps_misc", bufs=2, space="PSUM")
            )

            def d_group(tt, ec):
                ps = ps_misc.tile([P, 512], F32, tag="ps_misc")
                for h in range(HPC):
                    nc.tensor.matmul(
                        ps[:],
                        ctx_sb[h][:, tt * P : (tt + 1) * P],
                        wo_sb[h][:, ec * 512 : (ec + 1) * 512],
                        start=(h == 0),
                        stop=(h == HPC - 1),
                    )
                ob = opool.tile([P, 512], F16, tag="ob")
                # PSUM evacuation on DVE (scalar-engine psum copies measure
                # ~2x slower due to the read-write-bubble errata)
                nc.vector.tensor_copy(ob[:], ps[:])
                # out DMAs on gpsimd; the tail groups (after the last unit)
                # alternate gpsimd/scalar so the final drain uses two queues
                # (sync stays dedicated to the x-bar transposes)
                q = nc.gpsimd if (tt < 12 or (tt + ec) % 2 == 0) else nc.scalar
                q.dma_start(
                    out[tt * P : (tt + 1) * P, ec * 512 : (ec + 1) * 512], ob[:]
                )

            def unit(jb, h, d_take, pre):
                """One (q-block, head): scoresT+exp, PV, denominators, and
                normalized context write; d_take() interleaves out-proj work,
                pre[0]/pre[1] run the previous unit's deferred normalization
                stages at pair 0/1 so their PE instructions land with other
                work queued between them."""
                qs = q_sb[h][:, jb * 512 : (jb + 1) * 512]
                pbt = pbt_pool.tile([P, NN, 512], F16, tag="pbt", name="pbt")
                pbt_flat = pbt[:].rearrange("p n m -> p (n m)")
                acc_a = accw_pool.tile([P, 1024], F16, tag="acca", name="acca")
                acc_b = accw_pool.tile([P, 1024], F16, tag="accb", name="accb")
                psc = ps_pv.tile([P, 512], F32, tag="ps_pv")

                def pv(nt):
                    nc.tensor.matmul(
                        psc[:], vvs[h][:, nt, :], pbt[:, nt, :],
                        start=(nt == 0), stop=(nt == NN - 1),
                    )

                for pair in range(8):
                    if pair == 1:
                        pre[0]()
                    elif pair == 3:
                        pre[1]()
                    if pair < 4:
                        d_take()
                    ps = ps_s.tile([P, 1024], F32, tag="ps_s")
                    for sub in range(2):
                        nt = 2 * pair + sub
                        nc.tensor.matmul(
                            ps[:, sub * 512 : (sub + 1) * 512],
                            k_sb[h][:, nt * P : (nt + 1) * P],
                            qs,
                            start=True,
                            stop=True,
                        )
                    if with_mask:
                        mt = mpool.tile([P, 1024], F32, tag="mt")
                        for sub in range(2):
                            nt = 2 * pair + sub
                            nc.gpsimd.dma_start(
                                mt[:, sub * 512 : (sub + 1) * 512],
                                maskf[nt * P : (nt + 1) * P,
                                      jb * 512 : (jb + 1) * 512],
                            )
                        nc.vector.tensor_add(ps[:], ps[:], mt[:])
                    pslice = pbt_flat[:, pair * 1024 : (pair + 1) * 1024]
                    nc.scalar.activation(
                        pslice,
                        ps[:],
                        mybir.ActivationFunctionType.Exp,
                        scale=SCALE,
                    )
                    # denominator accumulation: DVE takes pairs 0+1, 2, 4, 6,
                    # 7 (fast engine finishes the chain); gpsimd takes 3+5
                    # via a 3-address add (its copies are slow, adds are ok)
                    if pair == 1:
                        p0 = pbt_flat[:, 0:1024]
                        nc.vector.tensor_add(acc_a[:], p0, pslice)
                    elif pair == 5:
                        p3 = pbt_flat[:, 3 * 1024 : 4 * 1024]
                        nc.gpsimd.tensor_add(acc_b[:], p3, pslice)
                    elif pair in (2, 4, 6, 7):
                        nc.vector.tensor_add(acc_a[:], acc_a[:], pslice)
                    if pair >= 1:
                        pv(2 * (pair - 1))
                        pv(2 * (pair - 1) + 1)
                pv(14)
                pv(15)

                # fold the two [P, 1024] accumulators to [P, 512] fp16; free
                # the PV bank via the ctmp copy promptly. The partition
                # reduction (one small matmul) and the rest of the
                # normalization are deferred into finish(), a unit later, so
                # the PE's in-order queue never blocks on this chain.
                acc_bs = stats.tile([P, 512], F16, tag="acc_bs")
                nc.gpsimd.tensor_add(
                    acc_bs[:], acc_b[:, 0:512], acc_b[:, 512:1024]
                )
                acc_as = stats.tile([P, 512], F16, tag="acc_as")
                nc.vector.tensor_add(
                    acc_as[:], acc_a[:, 0:512], acc_a[:, 512:1024]
                )
                acc_f16 = stats.tile([P, 512], F16, tag="acc_f16")
                nc.vector.tensor_add(acc_f16[:], acc_as[:], acc_bs[:])
                ctmp = ctmp_pool.tile([P, 512], F16, tag="ctmp", name="ctmp")
                nc.vector.tensor_copy(ctmp[:], psc[:])

                box = {}

                def fin1():
                    # partition-reduce the denominator accumulator
                    dpsf = ps_misc.tile([P, 512], F32, tag="ps_misc")
                    nc.tensor.matmul(
                        dpsf[0:1, :], ones_col[:], acc_f16[:],
                        start=True, stop=True,
                    )
                    den_f16 = stats.tile([1, 512], F16, tag="den_f16")
                    nc.vector.tensor_copy(den_f16[:], dpsf[0:1, :])
                    box["den"] = den_f16

                def fin2():
                    den_f16 = box["den"]
                    # broadcast den across partitions (4 tiny matmuls), then
                    # reciprocal on the [128, 4] layout (all partitions busy)
                    dtp = ps_misc.tile([P, 512], F32, tag="ps_misc")
                    for j in range(4):
                        nc.tensor.matmul(
                            dtp[:, j : j + 1],
                            den_f16[0:1, j * P : (j + 1) * P],
                            ones_one[:],
                            start=True,
                            stop=True,
                        )
                    rect = stats.tile([P, 4], F32, tag="rect")
                    nc.vector.tensor_copy(rect[:], dtp[:, 0:4])
                    rrec = stats.tile([P, 4], F32, tag="rrec")
                    nc.vector.reciprocal(rrec[:], rect[:])
                    cn = cnrm_pool.tile([P, 4, P], F16, tag="cnrm", name="cnrm")
                    nc.sync.dma_start(cn[:], ctmp[:], transpose=True)
                    for j in range(4):
                        nc.vector.tensor_scalar_mul(
                            cn[:, j, :], cn[:, j, :], rrec[:, j : j + 1]
                        )
                    ctx_dst = ctx_sb[h][:, jb * 512 : (jb + 1) * 512].rearrange(
                        "p (a b) -> p a b", a=4
                    )
                    nc.sync.dma_start(ctx_dst, cn[:], transpose=True)

                return fin1, fin2

            nofin = (lambda: None, lambda: None)
            pending = []
            for jb in range(4):
                # each unit runs the previous unit's deferred normalization
                # stages at its pairs 0/1, so ctx planes of block jb-1 are
                # complete before the units of jb+1 consume them via d_take
                d_list = (
                    [(tt, ec) for tt in range(4 * jb - 4, 4 * jb)
                     for ec in range(NE)]
                    if jb > 0 else []
                )
                di = [0]

                def d_take():
                    if di[0] < len(d_list):
                        d_group(*d_list[di[0]])
                        di[0] += 1

                for h in range(HPC):
                    pre = pending.pop(0) if pending else nofin
                    fins = unit(jb, h, d_take if h > 0 else (lambda: None), pre)
                    pending.append(fins)
                while di[0] < len(d_list):
                    d_group(*d_list[di[0]])
                    di[0] += 1
            for f1, f2 in pending:
                f1()
                f2()
            for tt in range(12, 16):
                for ec in range(NE):
                    d_group(tt, ec)


def _build(with_mask: bool):
    nc = bacc.Bacc("TRN2")
    xT = nc.dram_tensor("xT", [DIM, S], BF16, kind="ExternalInput")
    wqT = nc.dram_tensor("wqT", [DIM, DL], BF16, kind="ExternalInput")
    wkT = nc.dram_tensor("wkT", [DIM, DL], BF16, kind="ExternalInput")
    wvT = nc.dram_tensor("wvT", [DIM, DL], BF16, kind="ExternalInput")
    woT = nc.dram_tensor("woT", [DL, DIM], F16, kind="ExternalInput")
    maskf = (
        nc.dram_tensor("maskf", [S, S], F32, kind="ExternalInput")
        if with_mask
        else None
    )
    out = nc.dram_tensor("out", [S, DIM], F16, kind="ExternalOutput")
    with tile.TileContext(nc) as tc:
        _emit(nc, tc, xT, wqT, wkT, wvT, woT, maskf, out)
    nc.finalize()
    return nc


def _get_program(with_mask: bool):
    if with_mask not in _PROGRAM_CACHE:
        _PROGRAM_CACHE[with_mask] = _build(with_mask)
    return _PROGRAM_CACHE[with_mask]


def _prep_in_maps(x, mask, wq, wk, wv, wo, with_mask):
    bf = ml_dtypes.bfloat16
    f32 = np.float32
    xTs = [np.ascontiguousarray(x[b].T.astype(bf)) for b in range(B)]
    if with_mask:
        maskf = np.ascontiguousarray(mask[0, 0].T.astype(f32) / SCALE)
    in_maps = []
    for c in range(N_CORES):
        b = c // HGROUPS
        g = c % HGROUPS
        sl = slice(g * DL, (g + 1) * DL)
        m = {
            "xT": xTs[b],
            "wqT": np.ascontiguousarray(wq[sl, :].T.astype(bf)),
            "wkT": np.ascontiguousarray(wk[sl, :].T.astype(bf)),
            "wvT": np.ascontiguousarray(wv[sl, :].T.astype(bf)),
            "woT": np.ascontiguousarray(wo[:, sl].T.astype(np.float16)),
        }
        if with_mask:
            m["maskf"] = maskf
        in_maps.append(m)
    return in_maps


def run_sharded(x, mask, wq, wk, wv, wo, trace=False, trace_kwargs=None):
    """Run the SPMD kernel; returns (full_output, BassKernelResults)."""
    with_mask = bool(np.any(np.asarray(mask)))
    nc = _get_program(with_mask)
    in_maps = _prep_in_maps(
        np.asarray(x), np.asarray(mask), np.asarray(wq), np.asarray(wk),
        np.asarray(wv), np.asarray(wo), with_mask,
    )
    kw = {}
    if trace:
        kw["trace"] = True
        if trace_kwargs:
            kw["trace_kwargs"] = trace_kwargs
    res = run_bass_kernel_spmd(nc, in_maps, list(range(N_CORES)), **kw)
    out = np.zeros((B, S, DIM), np.float32)
    for c in range(N_CORES):
        out[c // HGROUPS] += res.results[c]["out"].astype(np.float32)
    return out, res


def kernel(**inputs):
    out, _ = run_sharded(
        inputs["x"], inputs["mask"], inputs["wq"], inputs["wk"], inputs["wv"],
        inputs["wo"],
    )
    return out


# revision 28
# speedup vs baseline: 1.2430x; 1.2061x over previous
"""Multi-head attention block (QKV proj -> softmax attention -> out proj) for
Trainium2, SPMD across 8 NeuronCores.

Sharding: batch (B=2) x head-groups (4 groups of 4 heads). Core c handles
batch c//4 and heads [4*(c%4), 4*(c%4)+4). Each core computes its partial
output contribution (context @ wo_slice.T); the host sums the 4 head-group
partials per batch (tensor-parallel row-sharded wo => the all-reduce is the
host-side gather).

All matmuls run in bf16 with fp32 PSUM accumulation. Softmax runs in fp32
out of PSUM (exp on the scalar engine). Softmax denominators are computed by
accumulating the exp'd probability tiles on the DVE (fp32) and reducing the
final [128, 512] accumulator over partitions with a single ones-vector
matmul per unit -- this keeps the PE free of the per-tile ones-matmuls that
cost ~78us in v1. The reciprocal runs on a [128, 4] layout (128 partitions
busy) instead of [1, 512] (one partition). Partial outputs are written fp16
(host accumulates in fp32).

Per-core kernel layout (everything [partition=128, free]):
  xT   [2048, 2048] bf16   x[b].T             (feature k on partitions)
  wqT/wkT/wvT [2048, 512]  w[heads_slice].T   (k on partitions)
  woT  [512, 2048]  bf16   wo[:, slice].T     (local d on partitions)
  out  [2048, 2048] fp16   partial output for batch b
"""

import sys

if "/opt/trn_rl_repo" not in sys.path:
    sys.path.insert(0, "/opt/trn_rl_repo")

from contextlib import ExitStack

import ml_dtypes
import numpy as np

import concourse.bacc as bacc
import concourse.tile as tile
from concourse import mybir
from concourse.bass_utils import run_bass_kernel_spmd

BF16 = mybir.dt.bfloat16
F16 = mybir.dt.float16
F32 = mybir.dt.float32

B, S, DIM = 2, 2048, 2048
HEADS, HD = 16, 128
P = 128
N_CORES = 8
HGROUPS = 4  # head groups (second shard axis is batch)
HPC = HEADS // HGROUPS  # heads per core = 4
DL = HPC * HD  # local head dims per core = 512
SCALE = 1.0 / float(np.sqrt(HD))

NK = DIM // P  # 16 contraction tiles for the projections
NM = S // 512  # 4 m-chunks (tokens)
NQ = S // P  # 16 q tiles
NN = S // P  # 16 kv tiles
NE = DIM // 512  # 4 output-dim chunks

_PROGRAM_CACHE = {}


def _emit(nc, tc, xT, wqT, wkT, wvT, woT, maskf, out):
    with_mask = maskf is not None
    with ExitStack() as octx:
        planes = octx.enter_context(tc.tile_pool(name="planes", bufs=1))
        q_sb = [planes.tile([P, S], BF16, tag=f"q{h}", name=f"q{h}") for h in range(HPC)]
        k_sb = [planes.tile([P, S], BF16, tag=f"k{h}", name=f"k{h}") for h in range(HPC)]
        ctx_sb = [planes.tile([P, S], F16, tag=f"ctx{h}", name=f"ctx{h}") for h in range(HPC)]

        vv_pool = octx.enter_context(tc.tile_pool(name="vv", bufs=1))
        vvs = [None] * HPC

        # ---------------- Phase 1: QKV projections ----------------
        with ExitStack() as ctx:
            wpool = ctx.enter_context(tc.tile_pool(name="wqkv", bufs=1))
            vT_sb = [wpool.tile([P, S], F16, tag=f"vt{h}", name=f"vt{h}")
                     for h in range(HPC)]
            w_sb = {}
            for name in ("q", "k", "v"):
                w_sb[name] = wpool.tile([P, NK * DL], BF16, tag=f"w{name}",
                                        name=f"w{name}")
            xpool = ctx.enter_context(tc.tile_pool(name="xt", bufs=2 * NK))
            pq = ctx.enter_context(tc.tile_pool(name="ps_qkv", bufs=4, space="PSUM"))

            for mc in range(NM):
                xts = []
                for kt in range(NK):
                    t = xpool.tile([P, 512], BF16, tag="xt")
                    nc.sync.dma_start(
                        t[:], xT[kt * P : (kt + 1) * P, mc * 512 : (mc + 1) * 512]
                    )
                    xts.append(t)
                    if mc == 0:
                        # k weights on the gpsimd queue in parallel with the
                        # x tiles on sync, so the first accumulation group
                        # starts as early as possible; v/q follow
                        nc.gpsimd.dma_start(
                            w_sb["k"][:, kt * DL : (kt + 1) * DL],
                            wkT[kt * P : (kt + 1) * P, :],
                        )
                if mc == 0:
                    for kt in range(NK):
                        for name, srct in (("v", wvT), ("q", wqT)):
                            nc.sync.dma_start(
                                w_sb[name][:, kt * DL : (kt + 1) * DL],
                                srct[kt * P : (kt + 1) * P, :],
                            )
                # k/v planes first so attention for head h can start (and the
                # vv transposes can issue) before the q planes of the last
                # m-chunk are finished
                plan = [(n, h) for h in range(HPC) for n in ("k", "v")]
                plan += [("q", h) for h in range(HPC)]
                for name, h in plan:
                    plane = {"q": q_sb, "k": k_sb, "v": vT_sb}[name][h]
                    ps = pq.tile([P, 512], F32, tag="ps")
                    for kt in range(NK):
                        nc.tensor.matmul(
                            ps[:],
                            w_sb[name][:, kt * DL + h * P : kt * DL + (h + 1) * P],
                            xts[kt][:],
                            start=(kt == 0),
                            stop=(kt == NK - 1),
                        )
                    nc.any.tensor_copy(plane[:, mc * 512 : (mc + 1) * 512], ps[:])
                    if name == "v" and mc == NM - 1:
                        # v plane complete: transpose to [kv, d] via DMA x-bar
                        vv = vv_pool.tile([P, NN, P], F16, tag=f"vv{h}",
                                          name=f"vv{h}")
                        nc.sync.dma_start(vv[:], vT_sb[h][:], transpose=True)
                        vvs[h] = vv

        # ------- Phase 2+3: attention (scoresT form) + out projection -------
        # scoresT(nt) = kT(nt)^T @ qT  ->  exp -> probsT [n, m] directly (no
        # transpose on the softmax path). Denominators: DVE accumulates the
        # exp'd tiles (fp32), one small ones-matmul per unit reduces over
        # partitions; normalization applied to the small context tensor
        # through a pair of x-bar transposes, off the critical path.
        with ExitStack() as ctx:
            wopool = ctx.enter_context(tc.tile_pool(name="wo", bufs=1))
            wo_sb = [wopool.tile([P, DIM], F16, tag=f"wo{h}", name=f"wo{h}")
                     for h in range(HPC)]
            for h in range(HPC):
                nc.gpsimd.dma_start(wo_sb[h][:], woT[h * P : (h + 1) * P, :])

            const = ctx.enter_context(tc.tile_pool(name="const", bufs=1))
            ones_col = const.tile([P, 1], F16, tag="ones_col")
            nc.any.memset(ones_col[:], 1.0)
            ones_one = const.tile([1, 1], F16, tag="ones_one")
            nc.any.memset(ones_one[:], 1.0)

            pbt_pool = ctx.enter_context(tc.tile_pool(name="pbt", bufs=3))
            accw_pool = ctx.enter_context(tc.tile_pool(name="accw", bufs=3))
            ctmp_pool = ctx.enter_context(tc.tile_pool(name="ctmp", bufs=3))
            cnrm_pool = ctx.enter_context(tc.tile_pool(name="cnrm", bufs=3))
            stats = ctx.enter_context(tc.tile_pool(name="stats", bufs=4))
            opool = ctx.enter_context(tc.tile_pool(name="ob", bufs=3))
            if with_mask:
                mpool = ctx.enter_context(tc.tile_pool(name="mask", bufs=4))
            # pool creation order fixes bank assignment: ps_pv/ps_misc take
            # banks 0-3 (aliasing phase-1's pq, consumed late) and ps_s gets
            # banks 4-7, free during phase 1, so the first units' score
            # matmuls can overlap the tail of the QKV projections
            ps_pv = ctx.enter_context(tc.tile_pool(name="ps_pv", bufs=2, space="PSUM"))
            ps_misc = ctx.enter_context(
                tc.tile_pool(name="ps_misc", bufs=2, space="PSUM")
            )
            ps_s = ctx.enter_context(tc.tile_pool(name="ps_s", bufs=2, space="PSUM"))

            dma_q = [nc.gpsimd, nc.sync]

            def d_group(tt, ec):
                ps = ps_misc.tile([P, 512], F32, tag="ps_misc")
                for h in range(HPC):
                    nc.tensor.matmul(
                        ps[:],
                        ctx_sb[h][:, tt * P : (tt + 1) * P],
                        wo_sb[h][:, ec * 512 : (ec + 1) * 512],
                        start=(h == 0),
                        stop=(h == HPC - 1),
                    )
                ob = opool.tile([P, 512], F16, tag="ob")
                # PSUM evacuation split between DVE and the scalar engine
                if (tt + ec) % 2 == 0:
                    nc.vector.tensor_copy(ob[:], ps[:])
                else:
                    nc.scalar.copy(ob[:], ps[:])
                dma_q[(tt + ec) % 2].dma_start(
                    out[tt * P : (tt + 1) * P, ec * 512 : (ec + 1) * 512], ob[:]
                )

            def unit(jb, h, d_take, pre):
                """One (q-block, head): scoresT+exp, PV, denominators, and
                normalized context write; d_take() interleaves out-proj work,
                pre[0]/pre[1] run the previous unit's deferred normalization
                stages at pair 0/1 so their PE instructions land with other
                work queued between them."""
                qs = q_sb[h][:, jb * 512 : (jb + 1) * 512]
                pbt = pbt_pool.tile([P, NN, 512], F16, tag="pbt", name="pbt")
                pbt_flat = pbt[:].rearrange("p n m -> p (n m)")
                acc_a = accw_pool.tile([P, 1024], F16, tag="acca", name="acca")
                acc_b = accw_pool.tile([P, 1024], F16, tag="accb", name="accb")
                psc = ps_pv.tile([P, 512], F32, tag="ps_pv")

                def pv(nt):
                    nc.tensor.matmul(
                        psc[:], vvs[h][:, nt, :], pbt[:, nt, :],
                        start=(nt == 0), stop=(nt == NN - 1),
                    )

                for pair in range(8):
                    if pair < 2:
                        pre[pair]()
                    if pair < 4:
                        d_take()
                    ps = ps_s.tile([P, 1024], F32, tag="ps_s")
                    for sub in range(2):
                        nt = 2 * pair + sub
                        nc.tensor.matmul(
                            ps[:, sub * 512 : (sub + 1) * 512],
                            k_sb[h][:, nt * P : (nt + 1) * P],
                            qs,
                            start=True,
                            stop=True,
                        )
                    if with_mask:
                        mt = mpool.tile([P, 1024], F32, tag="mt")
                        for sub in range(2):
                            nt = 2 * pair + sub
                            nc.gpsimd.dma_start(
                                mt[:, sub * 512 : (sub + 1) * 512],
                                maskf[nt * P : (nt + 1) * P,
                                      jb * 512 : (jb + 1) * 512],
                            )
                        nc.vector.tensor_add(ps[:], ps[:], mt[:])
                    pslice = pbt_flat[:, pair * 1024 : (pair + 1) * 1024]
                    nc.scalar.activation(
                        pslice,
                        ps[:],
                        mybir.ActivationFunctionType.Exp,
                        scale=SCALE,
                    )
                    # denominator accumulation: DVE takes pairs 0+1, 2, 4, 6,
                    # 7 (fast engine finishes the chain); gpsimd takes 3+5
                    # via a 3-address add (its copies are slow, adds are ok)
                    if pair == 1:
                        p0 = pbt_flat[:, 0:1024]
                        nc.vector.tensor_add(acc_a[:], p0, pslice)
                    elif pair == 5:
                        p3 = pbt_flat[:, 3 * 1024 : 4 * 1024]
                        nc.gpsimd.tensor_add(acc_b[:], p3, pslice)
                    elif pair in (2, 4, 6, 7):
                        nc.vector.tensor_add(acc_a[:], acc_a[:], pslice)
                    if pair >= 1:
                        pv(2 * (pair - 1))
                        pv(2 * (pair - 1) + 1)
                pv(14)
                pv(15)

                # fold the two [P, 1024] accumulators to [P, 512] fp16; free
                # the PV bank via the ctmp copy promptly. The partition
                # reduction (one small matmul) and the rest of the
                # normalization are deferred into finish(), a unit later, so
                # the PE's in-order queue never blocks on this chain.
                acc_bs = stats.tile([P, 512], F16, tag="acc_bs")
                nc.gpsimd.tensor_add(
                    acc_bs[:], acc_b[:, 0:512], acc_b[:, 512:1024]
                )
                acc_as = stats.tile([P, 512], F16, tag="acc_as")
                nc.vector.tensor_add(
                    acc_as[:], acc_a[:, 0:512], acc_a[:, 512:1024]
                )
                acc_f16 = stats.tile([P, 512], F16, tag="acc_f16")
                nc.vector.tensor_add(acc_f16[:], acc_as[:], acc_bs[:])
                ctmp = ctmp_pool.tile([P, 512], F16, tag="ctmp", name="ctmp")
                nc.vector.tensor_copy(ctmp[:], psc[:])

                box = {}

                def fin1():
                    # partition-reduce the denominator accumulator
                    dpsf = ps_misc.tile([P, 512], F32, tag="ps_misc")
                    nc.tensor.matmul(
                        dpsf[0:1, :], ones_col[:], acc_f16[:],
                        start=True, stop=True,
                    )
                    den_f16 = stats.tile([1, 512], F16, tag="den_f16")
                    nc.vector.tensor_copy(den_f16[:], dpsf[0:1, :])
                    box["den"] = den_f16

                def fin2():
                    den_f16 = box["den"]
                    # broadcast den across partitions (4 tiny matmuls), then
                    # reciprocal on the [128, 4] layout (all partitions busy)
                    dtp = ps_misc.tile([P, 512], F32, tag="ps_misc")
                    for j in range(4):
                        nc.tensor.matmul(
                            dtp[:, j : j + 1],
                            den_f16[0:1, j * P : (j + 1) * P],
                            ones_one[:],
                            start=True,
                            stop=True,
                        )
                    rect = stats.tile([P, 4], F32, tag="rect")
                    nc.vector.tensor_copy(rect[:], dtp[:, 0:4])
                    rrec = stats.tile([P, 4], F32, tag="rrec")
                    nc.vector.reciprocal(rrec[:], rect[:])
                    cn = cnrm_pool.tile([P, 4, P], F16, tag="cnrm", name="cnrm")
                    nc.sync.dma_start(cn[:], ctmp[:], transpose=True)
                    for j in range(4):
                        nc.vector.tensor_scalar_mul(
                            cn[:, j, :], cn[:, j, :], rrec[:, j : j + 1]
                        )
                    ctx_dst = ctx_sb[h][:, jb * 512 : (jb + 1) * 512].rearrange(
                        "p (a b) -> p a b", a=4
                    )
                    nc.sync.dma_start(ctx_dst, cn[:], transpose=True)

                return fin1, fin2

            nofin = (lambda: None, lambda: None)
            pending = []
            for jb in range(4):
                # each unit runs the previous unit's deferred normalization
                # stages at its pairs 0/1, so ctx planes of block jb-1 are
                # complete before the units of jb+1 consume them via d_take
                d_list = (
                    [(tt, ec) for tt in range(4 * jb - 4, 4 * jb)
                     for ec in range(NE)]
                    if jb > 0 else []
                )
                di = [0]

                def d_take():
                    if di[0] < len(d_list):
                        d_group(*d_list[di[0]])
                        di[0] += 1

                for h in range(HPC):
                    pre = pending.pop(0) if pending else nofin
                    fins = unit(jb, h, d_take if h > 0 else (lambda: None), pre)
                    pending.append(fins)
                while di[0] < len(d_list):
                    d_group(*d_list[di[0]])
                    di[0] += 1
            for f1, f2 in pending:
                f1()
                f2()
            for tt in range(12, 16):
                for ec in range(NE):
                    d_group(tt, ec)


def _build(with_mask: bool):
    nc = bacc.Bacc("TRN2")
    xT = nc.dram_tensor("xT", [DIM, S], BF16, kind="ExternalInput")
    wqT = nc.dram_tensor("wqT", [DIM, DL], BF16, kind="ExternalInput")
    wkT = nc.dram_tensor("wkT", [DIM, DL], BF16, kind="ExternalInput")
    wvT = nc.dram_tensor("wvT", [DIM, DL], BF16, kind="ExternalInput")
    woT = nc.dram_tensor("woT", [DL, DIM], F16, kind="ExternalInput")
    maskf = (
        nc.dram_tensor("maskf", [S, S], F32, kind="ExternalInput")
        if with_mask
        else None
    )
    out = nc.dram_tensor("out", [S, DIM], F16, kind="ExternalOutput")
    with tile.TileContext(nc) as tc:
        _emit(nc, tc, xT, wqT, wkT, wvT, woT, maskf, out)
    nc.finalize()
    return nc


def _get_program(with_mask: bool):
    if with_mask not in _PROGRAM_CACHE:
        _PROGRAM_CACHE[with_mask] = _build(with_mask)
    return _PROGRAM_CACHE[with_mask]


def _prep_in_maps(x, mask, wq, wk, wv, wo, with_mask):
    bf = ml_dtypes.bfloat16
    f32 = np.float32
    xTs = [np.ascontiguousarray(x[b].T.astype(bf)) for b in range(B)]
    if with_mask:
        maskf = np.ascontiguousarray(mask[0, 0].T.astype(f32) / SCALE)
    in_maps = []
    for c in range(N_CORES):
        b = c // HGROUPS
        g = c % HGROUPS
        sl = slice(g * DL, (g + 1) * DL)
        m = {
            "xT": xTs[b],
            "wqT": np.ascontiguousarray(wq[sl, :].T.astype(bf)),
            "wkT": np.ascontiguousarray(wk[sl, :].T.astype(bf)),
            "wvT": np.ascontiguousarray(wv[sl, :].T.astype(bf)),
            "woT": np.ascontiguousarray(wo[:, sl].T.astype(np.float16)),
        }
        if with_mask:
            m["maskf"] = maskf
        in_maps.append(m)
    return in_maps


def run_sharded(x, mask, wq, wk, wv, wo, trace=False, trace_kwargs=None):
    """Run the SPMD kernel; returns (full_output, BassKernelResults)."""
    with_mask = bool(np.any(np.asarray(mask)))
    nc = _get_program(with_mask)
    in_maps = _prep_in_maps(
        np.asarray(x), np.asarray(mask), np.asarray(wq), np.asarray(wk),
        np.asarray(wv), np.asarray(wo), with_mask,
    )
    kw = {}
    if trace:
        kw["trace"] = True
        if trace_kwargs:
            kw["trace_kwargs"] = trace_kwargs
    res = run_bass_kernel_spmd(nc, in_maps, list(range(N_CORES)), **kw)
    out = np.zeros((B, S, DIM), np.float32)
    for c in range(N_CORES):
        out[c // HGROUPS] += res.results[c]["out"].astype(np.float32)
    return out, res


def kernel(**inputs):
    out, _ = run_sharded(
        inputs["x"], inputs["mask"], inputs["wq"], inputs["wk"], inputs["wv"],
        inputs["wo"],
    )
    return out


# revision 32
# speedup vs baseline: 1.2735x; 1.0246x over previous
"""Multi-head attention block (QKV proj -> softmax attention -> out proj) for
Trainium2, SPMD across 8 NeuronCores.

Sharding: batch (B=2) x head-groups (4 groups of 4 heads). Core c handles
batch c//4 and heads [4*(c%4), 4*(c%4)+4). Each core computes its partial
output contribution (context @ wo_slice.T); the host sums the 4 head-group
partials per batch (tensor-parallel row-sharded wo => the all-reduce is the
host-side gather).

All matmuls run in bf16 with fp32 PSUM accumulation. Softmax runs in fp32
out of PSUM (exp on the scalar engine). Softmax denominators are computed by
accumulating the exp'd probability tiles on the DVE (fp32) and reducing the
final [128, 512] accumulator over partitions with a single ones-vector
matmul per unit -- this keeps the PE free of the per-tile ones-matmuls that
cost ~78us in v1. The reciprocal runs on a [128, 4] layout (128 partitions
busy) instead of [1, 512] (one partition). Partial outputs are written fp16
(host accumulates in fp32).

Per-core kernel layout (everything [partition=128, free]):
  xT   [2048, 2048] bf16   x[b].T             (feature k on partitions)
  wqT/wkT/wvT [2048, 512]  w[heads_slice].T   (k on partitions)
  woT  [512, 2048]  bf16   wo[:, slice].T     (local d on partitions)
  out  [2048, 2048] fp16   partial output for batch b
"""

import sys

if "/opt/trn_rl_repo" not in sys.path:
    sys.path.insert(0, "/opt/trn_rl_repo")

from contextlib import ExitStack

import ml_dtypes
import numpy as np

import concourse.bacc as bacc
import concourse.tile as tile
from concourse import mybir
from concourse.bass_utils import run_bass_kernel_spmd

BF16 = mybir.dt.bfloat16
F16 = mybir.dt.float16
F32 = mybir.dt.float32

B, S, DIM = 2, 2048, 2048
HEADS, HD = 16, 128
P = 128
N_CORES = 8
HGROUPS = 4  # head groups (second shard axis is batch)
HPC = HEADS // HGROUPS  # heads per core = 4
DL = HPC * HD  # local head dims per core = 512
SCALE = 1.0 / float(np.sqrt(HD))

NK = DIM // P  # 16 contraction tiles for the projections
NM = S // 512  # 4 m-chunks (tokens)
NQ = S // P  # 16 q tiles
NN = S // P  # 16 kv tiles
NE = DIM // 512  # 4 output-dim chunks

_PROGRAM_CACHE = {}


def _emit(nc, tc, xT, wqT, wkT, wvT, woT, maskf, out):
    with_mask = maskf is not None
    with ExitStack() as octx:
        planes = octx.enter_context(tc.tile_pool(name="planes", bufs=1))
        q_sb = [planes.tile([P, S], BF16, tag=f"q{h}", name=f"q{h}") for h in range(HPC)]
        k_sb = [planes.tile([P, S], BF16, tag=f"k{h}", name=f"k{h}") for h in range(HPC)]
        ctx_sb = [planes.tile([P, S], F16, tag=f"ctx{h}", name=f"ctx{h}") for h in range(HPC)]

        vv_pool = octx.enter_context(tc.tile_pool(name="vv", bufs=1))
        vvs = [None] * HPC

        # ---------------- Phase 1: QKV projections ----------------
        with ExitStack() as ctx:
            wpool = ctx.enter_context(tc.tile_pool(name="wqkv", bufs=1))
            vT_sb = [wpool.tile([P, S], F16, tag=f"vt{h}", name=f"vt{h}")
                     for h in range(HPC)]
            w_sb = {}
            for name in ("q", "k", "v"):
                w_sb[name] = wpool.tile([P, NK * DL], BF16, tag=f"w{name}",
                                        name=f"w{name}")
            xpool = ctx.enter_context(tc.tile_pool(name="xt", bufs=2 * NK))
            pq = ctx.enter_context(tc.tile_pool(name="ps_qkv", bufs=4, space="PSUM"))

            for mc in range(NM):
                xts = []
                for kt in range(NK):
                    t = xpool.tile([P, 512], BF16, tag="xt")
                    nc.sync.dma_start(
                        t[:], xT[kt * P : (kt + 1) * P, mc * 512 : (mc + 1) * 512]
                    )
                    xts.append(t)
                    if mc == 0:
                        # k weights on the gpsimd queue in parallel with the
                        # x tiles on sync, so the first accumulation group
                        # starts as early as possible; v/q follow
                        nc.gpsimd.dma_start(
                            w_sb["k"][:, kt * DL : (kt + 1) * DL],
                            wkT[kt * P : (kt + 1) * P, :],
                        )
                if mc == 0:
                    for kt in range(NK):
                        for name, srct in (("v", wvT), ("q", wqT)):
                            nc.sync.dma_start(
                                w_sb[name][:, kt * DL : (kt + 1) * DL],
                                srct[kt * P : (kt + 1) * P, :],
                            )
                # k/v planes first so attention for head h can start (and the
                # vv transposes can issue) before the q planes of the last
                # m-chunk are finished
                plan = [(n, h) for h in range(HPC) for n in ("k", "v")]
                plan += [("q", h) for h in range(HPC)]
                for name, h in plan:
                    plane = {"q": q_sb, "k": k_sb, "v": vT_sb}[name][h]
                    ps = pq.tile([P, 512], F32, tag="ps")
                    for kt in range(NK):
                        nc.tensor.matmul(
                            ps[:],
                            w_sb[name][:, kt * DL + h * P : kt * DL + (h + 1) * P],
                            xts[kt][:],
                            start=(kt == 0),
                            stop=(kt == NK - 1),
                        )
                    nc.any.tensor_copy(plane[:, mc * 512 : (mc + 1) * 512], ps[:])
                    if name == "v" and mc == NM - 1:
                        # v plane complete: transpose to [kv, d] via DMA x-bar
                        vv = vv_pool.tile([P, NN, P], F16, tag=f"vv{h}",
                                          name=f"vv{h}")
                        nc.sync.dma_start(vv[:], vT_sb[h][:], transpose=True)
                        vvs[h] = vv

        # ------- Phase 2+3: attention (scoresT form) + out projection -------
        # scoresT(nt) = kT(nt)^T @ qT  ->  exp -> probsT [n, m] directly (no
        # transpose on the softmax path). Denominators: DVE accumulates the
        # exp'd tiles (fp32), one small ones-matmul per unit reduces over
        # partitions; normalization applied to the small context tensor
        # through a pair of x-bar transposes, off the critical path.
        with ExitStack() as ctx:
            wopool = ctx.enter_context(tc.tile_pool(name="wo", bufs=1))
            wo_sb = [wopool.tile([P, DIM], F16, tag=f"wo{h}", name=f"wo{h}")
                     for h in range(HPC)]
            for h in range(HPC):
                nc.gpsimd.dma_start(wo_sb[h][:], woT[h * P : (h + 1) * P, :])

            const = ctx.enter_context(tc.tile_pool(name="const", bufs=1))
            ones_col = const.tile([P, 1], F16, tag="ones_col")
            nc.any.memset(ones_col[:], 1.0)
            ones_one = const.tile([1, 1], F16, tag="ones_one")
            nc.any.memset(ones_one[:], 1.0)

            pbt_pool = ctx.enter_context(tc.tile_pool(name="pbt", bufs=3))
            accw_pool = ctx.enter_context(tc.tile_pool(name="accw", bufs=3))
            ctmp_pool = ctx.enter_context(tc.tile_pool(name="ctmp", bufs=3))
            cnrm_pool = ctx.enter_context(tc.tile_pool(name="cnrm", bufs=3))
            stats = ctx.enter_context(tc.tile_pool(name="stats", bufs=4))
            opool = ctx.enter_context(tc.tile_pool(name="ob", bufs=3))
            if with_mask:
                mpool = ctx.enter_context(tc.tile_pool(name="mask", bufs=4))
            # pool creation order fixes bank assignment: ps_pv/ps_misc take
            # banks 0-3 (aliasing phase-1's pq, consumed late) and ps_s gets
            # banks 4-7, free during phase 1, so the first units' score
            # matmuls can overlap the tail of the QKV projections
            ps_pv = ctx.enter_context(tc.tile_pool(name="ps_pv", bufs=2, space="PSUM"))
            ps_misc = ctx.enter_context(
                tc.tile_pool(name="ps_misc", bufs=2, space="PSUM")
            )
            ps_s = ctx.enter_context(tc.tile_pool(name="ps_s", bufs=2, space="PSUM"))

            dma_q = [nc.gpsimd, nc.sync]

            def d_group(tt, ec):
                ps = ps_misc.tile([P, 512], F32, tag="ps_misc")
                for h in range(HPC):
                    nc.tensor.matmul(
                        ps[:],
                        ctx_sb[h][:, tt * P : (tt + 1) * P],
                        wo_sb[h][:, ec * 512 : (ec + 1) * 512],
                        start=(h == 0),
                        stop=(h == HPC - 1),
                    )
                ob = opool.tile([P, 512], F16, tag="ob")
                # PSUM evacuation split between DVE and the scalar engine
                if (tt + ec) % 2 == 0:
                    nc.vector.tensor_copy(ob[:], ps[:])
                else:
                    nc.scalar.copy(ob[:], ps[:])
                dma_q[(tt + ec) % 2].dma_start(
                    out[tt * P : (tt + 1) * P, ec * 512 : (ec + 1) * 512], ob[:]
                )

            def unit(jb, h, d_take, pre):
                """One (q-block, head): scoresT+exp, PV, denominators, and
                normalized context write; d_take() interleaves out-proj work,
                pre[0]/pre[1] run the previous unit's deferred normalization
                stages at pair 0/1 so their PE instructions land with other
                work queued between them."""
                qs = q_sb[h][:, jb * 512 : (jb + 1) * 512]
                pbt = pbt_pool.tile([P, NN, 512], F16, tag="pbt", name="pbt")
                pbt_flat = pbt[:].rearrange("p n m -> p (n m)")
                acc_a = accw_pool.tile([P, 1024], F16, tag="acca", name="acca")
                acc_b = accw_pool.tile([P, 1024], F16, tag="accb", name="accb")
                psc = ps_pv.tile([P, 512], F32, tag="ps_pv", bufs=1)

                def pv(nt):
                    nc.tensor.matmul(
                        psc[:], vvs[h][:, nt, :], pbt[:, nt, :],
                        start=(nt == 0), stop=(nt == NN - 1),
                    )

                for pair in range(8):
                    if pair < 4:
                        d_take()
                    # deferred stages after d_take: the d_group matmuls give
                    # the PE queue work while the den chain's DVE ops land
                    if pair == 1:
                        pre[0]()
                    elif pair == 3:
                        pre[1]()
                    ps = ps_s.tile([P, 1024], F32, tag="ps_s")
                    for sub in range(2):
                        nt = 2 * pair + sub
                        nc.tensor.matmul(
                            ps[:, sub * 512 : (sub + 1) * 512],
                            k_sb[h][:, nt * P : (nt + 1) * P],
                            qs,
                            start=True,
                            stop=True,
                        )
                    if with_mask:
                        mt = mpool.tile([P, 1024], F32, tag="mt")
                        for sub in range(2):
                            nt = 2 * pair + sub
                            nc.gpsimd.dma_start(
                                mt[:, sub * 512 : (sub + 1) * 512],
                                maskf[nt * P : (nt + 1) * P,
                                      jb * 512 : (jb + 1) * 512],
                            )
                        nc.vector.tensor_add(ps[:], ps[:], mt[:])
                    pslice = pbt_flat[:, pair * 1024 : (pair + 1) * 1024]
                    nc.scalar.activation(
                        pslice,
                        ps[:],
                        mybir.ActivationFunctionType.Exp,
                        scale=SCALE,
                    )
                    # denominator accumulation: DVE takes pairs 0+1, 2, 4, 6,
                    # 7 (fast engine finishes the chain); gpsimd takes 3+5
                    # via a 3-address add (its copies are slow, adds are ok)
                    if pair == 1:
                        p0 = pbt_flat[:, 0:1024]
                        nc.vector.tensor_add(acc_a[:], p0, pslice)
                    elif pair == 5:
                        p3 = pbt_flat[:, 3 * 1024 : 4 * 1024]
                        nc.gpsimd.tensor_add(acc_b[:], p3, pslice)
                    elif pair in (2, 4, 6, 7):
                        nc.vector.tensor_add(acc_a[:], acc_a[:], pslice)
                    if pair >= 1:
                        pv(2 * (pair - 1))
                        pv(2 * (pair - 1) + 1)
                pv(14)
                pv(15)

                # fold the two [P, 1024] accumulators to [P, 512] fp16; free
                # the PV bank via the ctmp copy promptly. The partition
                # reduction (one small matmul) and the rest of the
                # normalization are deferred into finish(), a unit later, so
                # the PE's in-order queue never blocks on this chain.
                acc_bs = stats.tile([P, 512], F16, tag="acc_bs")
                nc.gpsimd.tensor_add(
                    acc_bs[:], acc_b[:, 0:512], acc_b[:, 512:1024]
                )
                acc_as = stats.tile([P, 512], F16, tag="acc_as")
                nc.vector.tensor_add(
                    acc_as[:], acc_a[:, 0:512], acc_a[:, 512:1024]
                )
                acc_f16 = stats.tile([P, 512], F16, tag="acc_f16")
                nc.vector.tensor_add(acc_f16[:], acc_as[:], acc_bs[:])
                ctmp = ctmp_pool.tile([P, 512], F16, tag="ctmp", name="ctmp")
                nc.vector.tensor_copy(ctmp[:], psc[:])

                box = {}

                def fin1():
                    # partition-reduce the denominator accumulator; den psum
                    # lives in its own bank (ps_pv pool, "dps" ring) so it
                    # does not compete with the d_group banks
                    dpsf = ps_pv.tile([P, 512], F32, tag="dps", bufs=1)
                    nc.tensor.matmul(
                        dpsf[0:1, :], ones_col[:], acc_f16[:],
                        start=True, stop=True,
                    )
                    den_f16 = stats.tile([1, 512], F16, tag="den_f16")
                    nc.vector.tensor_copy(den_f16[:], dpsf[0:1, :])
                    box["den"] = den_f16

                def fin2():
                    den_f16 = box["den"]
                    # broadcast den across partitions (4 tiny matmuls), then
                    # reciprocal on the [128, 4] layout (all partitions busy)
                    dtp = ps_pv.tile([P, 512], F32, tag="dps", bufs=1)
                    for j in range(4):
                        nc.tensor.matmul(
                            dtp[:, j : j + 1],
                            den_f16[0:1, j * P : (j + 1) * P],
                            ones_one[:],
                            start=True,
                            stop=True,
                        )
                    rect = stats.tile([P, 4], F32, tag="rect")
                    nc.vector.tensor_copy(rect[:], dtp[:, 0:4])
                    rrec = stats.tile([P, 4], F32, tag="rrec")
                    nc.vector.reciprocal(rrec[:], rect[:])
                    cn = cnrm_pool.tile([P, 4, P], F16, tag="cnrm", name="cnrm")
                    nc.sync.dma_start(cn[:], ctmp[:], transpose=True)
                    for j in range(4):
                        nc.vector.tensor_scalar_mul(
                            cn[:, j, :], cn[:, j, :], rrec[:, j : j + 1]
                        )
                    ctx_dst = ctx_sb[h][:, jb * 512 : (jb + 1) * 512].rearrange(
                        "p (a b) -> p a b", a=4
                    )
                    nc.sync.dma_start(ctx_dst, cn[:], transpose=True)

                return fin1, fin2

            nofin = (lambda: None, lambda: None)
            pending = []
            for jb in range(4):
                # each unit runs the previous unit's deferred normalization
                # stages at its pairs 0/1, so ctx planes of block jb-1 are
                # complete before the units of jb+1 consume them via d_take
                d_list = (
                    [(tt, ec) for tt in range(4 * jb - 4, 4 * jb)
                     for ec in range(NE)]
                    if jb > 0 else []
                )
                di = [0]

                def d_take():
                    if di[0] < len(d_list):
                        d_group(*d_list[di[0]])
                        di[0] += 1

                for h in range(HPC):
                    pre = pending.pop(0) if pending else nofin
                    fins = unit(jb, h, d_take if h > 0 else (lambda: None), pre)
                    pending.append(fins)
                while di[0] < len(d_list):
                    d_group(*d_list[di[0]])
                    di[0] += 1
            for f1, f2 in pending:
                f1()
                f2()
            for tt in range(12, 16):
                for ec in range(NE):
                    d_group(tt, ec)


def _build(with_mask: bool):
    nc = bacc.Bacc("TRN2")
    xT = nc.dram_tensor("xT", [DIM, S], BF16, kind="ExternalInput")
    wqT = nc.dram_tensor("wqT", [DIM, DL], BF16, kind="ExternalInput")
    wkT = nc.dram_tensor("wkT", [DIM, DL], BF16, kind="ExternalInput")
    wvT = nc.dram_tensor("wvT", [DIM, DL], BF16, kind="ExternalInput")
    woT = nc.dram_tensor("woT", [DL, DIM], F16, kind="ExternalInput")
    maskf = (
        nc.dram_tensor("maskf", [S, S], F32, kind="ExternalInput")
        if with_mask
        else None
    )
    out = nc.dram_tensor("out", [S, DIM], F16, kind="ExternalOutput")
    with tile.TileContext(nc) as tc:
        _emit(nc, tc, xT, wqT, wkT, wvT, woT, maskf, out)
    nc.finalize()
    return nc


def _get_program(with_mask: bool):
    if with_mask not in _PROGRAM_CACHE:
        _PROGRAM_CACHE[with_mask] = _build(with_mask)
    return _PROGRAM_CACHE[with_mask]


def _prep_in_maps(x, mask, wq, wk, wv, wo, with_mask):
    bf = ml_dtypes.bfloat16
    f32 = np.float32
    xTs = [np.ascontiguousarray(x[b].T.astype(bf)) for b in range(B)]
    if with_mask:
        maskf = np.ascontiguousarray(mask[0, 0].T.astype(f32) / SCALE)
    in_maps = []
    for c in range(N_CORES):
        b = c // HGROUPS
        g = c % HGROUPS
        sl = slice(g * DL, (g + 1) * DL)
        m = {
            "xT": xTs[b],
            "wqT": np.ascontiguousarray(wq[sl, :].T.astype(bf)),
            "wkT": np.ascontiguousarray(wk[sl, :].T.astype(bf)),
            "wvT": np.ascontiguousarray(wv[sl, :].T.astype(bf)),
            "woT": np.ascontiguousarray(wo[:, sl].T.astype(np.float16)),
        }
        if with_mask:
            m["maskf"] = maskf
        in_maps.append(m)
    return in_maps


def run_sharded(x, mask, wq, wk, wv, wo, trace=False, trace_kwargs=None):
    """Run the SPMD kernel; returns (full_output, BassKernelResults)."""
    with_mask = bool(np.any(np.asarray(mask)))
    nc = _get_program(with_mask)
    in_maps = _prep_in_maps(
        np.asarray(x), np.asarray(mask), np.asarray(wq), np.asarray(wk),
        np.asarray(wv), np.asarray(wo), with_mask,
    )
    kw = {}
    if trace:
        kw["trace"] = True
        if trace_kwargs:
            kw["trace_kwargs"] = trace_kwargs
    res = run_bass_kernel_spmd(nc, in_maps, list(range(N_CORES)), **kw)
    out = np.zeros((B, S, DIM), np.float32)
    for c in range(N_CORES):
        out[c // HGROUPS] += res.results[c]["out"].astype(np.float32)
    return out, res


def kernel(**inputs):
    out, _ = run_sharded(
        inputs["x"], inputs["mask"], inputs["wq"], inputs["wk"], inputs["wv"],
        inputs["wo"],
    )
    return out
